# revision 29
# baseline (speedup 1.0000x reference)
"""DiversityAttention on 8 TRN2 NeuronCores (Bass/Tile).

Sharding: data-parallel over batch (B=2) x tensor-parallel over heads
(16 heads -> 4 groups of 4). core = (b, g), b = core // 4, g = core % 4.
Each core computes full attention for its 4 heads over its batch and a
partial out-projection [S, HIDDEN]; the host sums the 4 partials per
batch and adds bo.

The PE streams ~1 output column per cycle globally (tile-position
concurrency does not multiply throughput), so the kernel minimizes
total streamed columns. Q/K projections and scores run bf16; the sim
term, probabilities (pt) and V run fp8 DoubleRow, which contracts 256
rows per instruction and thus halves the ctx and sums column counts.

Device formulation, keys-on-partitions ("S^T") orientation:
  qT = (Wq/8 @ xb + bq/8)  [128(2h*64), pair, S]   bf16
  kT = (Wk @ xb + bk)      likewise
  V8 [keys%128, ktp, i, h, 64]  fp8 (i = kt sub-block of the pair)
  per (qb, ktp) over kt pairs (i = 0, 1):
    sim_ps_i = xq8^T xq8 (fp8 DoubleRow)           psum f32
    Ep[:,i]  = exp(-gamma * sim_ps_i)  (ACT)       bf16
    per head h: sc_h = kT^T qT (2 matmuls, K=64)   psum [128, 2, QB]
      pexp_h = exp(sc_h)               (ACT)       bf16
      pt8_h  = pexp_h * Ep             (DVE)       fp8 [128, 2, QB]
    lagged per-head pops: ctx_j += V8^T pt8 (DoubleRow, K=256)
                          sums  += ones8^T pt8 (DoubleRow, M=1)
  division: one reciprocal over the 4 strided sums rows, K=1 fp32
  broadcast matmuls, DVE mul -> ctxT2 bf16; out-projection of the
  previous query block is interleaved into the current block's loop.
"""

import math
import os
import sys

import numpy as np

for _p in ("/opt/trn_rl_repo",):
    if _p not in sys.path and os.path.isdir(_p):
        sys.path.insert(0, _p)

os.environ.setdefault("MYCRO_LOCAL_CACHE", "1")

import ml_dtypes

import concourse.bass as bass
import concourse.tile as tile
from concourse import bacc, mybir
from concourse.bass_utils import run_bass_kernel_spmd
from concourse.masks import make_identity


def _install_ntff_hook():
    """Provide antenv.axon_hooks (NTFF profiling registry) if the image
    lacks it, mirroring trn_agent_boot's ctypes hook. No-op on failure."""
    try:
        import antenv.axon_hooks  # noqa: F401
        return
    except ImportError:
        pass
    try:
        import contextlib
        import ctypes
        import types

        so_path = "/opt/axon/libaxon_pjrt.so"
        if not os.path.exists(so_path):
            return
        lib = ctypes.CDLL(so_path)
        if not hasattr(lib, "axon_start_nrt_profile"):
            return
        lib.axon_start_nrt_profile.argtypes = [
            ctypes.POINTER(ctypes.c_int64), ctypes.c_size_t]
        lib.axon_start_nrt_profile.restype = ctypes.c_int64
        lib.axon_stop_nrt_profile.argtypes = [ctypes.c_char_p]
        lib.axon_stop_nrt_profile.restype = ctypes.c_int64

        @contextlib.contextmanager
        def _hook(output_dir, device_ids):
            import jax
            jax.devices()
            if device_ids:
                ids = (ctypes.c_int64 * len(device_ids))(*device_ids)
                rc = lib.axon_start_nrt_profile(ids, len(device_ids))
            else:
                rc = lib.axon_start_nrt_profile(None, 0)
            if rc != 0:
                raise RuntimeError(f"axon_start_nrt_profile rc={rc}")
            try:
                yield
            finally:
                n = lib.axon_stop_nrt_profile(str(output_dir).encode())
                print(f"ntff profile: {n} file(s) -> {output_dir}",
                      file=sys.stderr)

        mod = types.ModuleType("antenv.axon_hooks")
        _state = {"hook": _hook}
        mod.set_axon_ntff_profile_hook = lambda h: _state.__setitem__("hook", h)
        mod.get_axon_ntff_profile_hook = lambda: _state["hook"]
        sys.modules["antenv.axon_hooks"] = mod
        import antenv
        antenv.axon_hooks = mod
    except Exception:
        pass


_install_ntff_hook()

F32 = mybir.dt.float32
BF16 = mybir.dt.bfloat16
FP8 = mybir.dt.float8e4
XQ8_SCALE = 16.0
ACT_EXP = mybir.ActivationFunctionType.Exp
ACT_COPY = mybir.ActivationFunctionType.Copy
ACT_IDENT = mybir.ActivationFunctionType.Identity
ALU = mybir.AluOpType

# Problem constants (hardcoded per contract).
HIDDEN = 1024
HEADS = 16
HEAD_DIM = 64
GAMMA = 0.5
B, S = 2, 2048
N_CORES = 8
GROUPS = N_CORES // B   # head groups per batch
HPC = HEADS // GROUPS   # heads per core
PAIRS = HPC // 2
CT = HIDDEN // 128      # contraction tiles
QB = 512
NQB = S // QB
NKT = S // 128
NKTP = NKT // 2         # kt pairs (DoubleRow ctx/sums contract 256 keys)
LAG = 2                 # kt lag between pt and ctx matmul
MASK_BIG = 60.0         # additive mask magnitude inside exp
DR = mybir.MatmulPerfMode.DoubleRow
F32R = mybir.dt.float32r
# E carries an extra 1/64 so pt8 = pexp*E stays inside fp8e4's range
# (score tails reach ~8.7 -> pexp ~6000; 6000*e^0.5/64 = 155 < 240);
# the softmax division cancels the scale exactly.
PT8_LBIAS = -math.log(64.0)


def emit_kernel(tc, aps):
    nc = tc.nc

    xb_d = aps["xb"]; xq8_d = aps["xq8"]
    wq_d = aps["wq"]; wk_d = aps["wk"]; wv_d = aps["wv"]; wo_d = aps["wo"]
    bq_d = aps["bq"]; bk_d = aps["bk"]; bv_d = aps["bv"]
    out_d = aps["out"]
    mask_d = aps.get("maskadd")

    from contextlib import ExitStack
    stack = ExitStack()
    consts = stack.enter_context(tc.tile_pool(name="consts", bufs=1))

    ones64 = consts.tile([128, 64], BF16)
    nc.vector.memset(ones64, 1.0)
    ident64 = consts.tile([64, 64], BF16)
    make_identity(nc, ident64)

    xb_sb = consts.tile([128, CT, S], BF16)
    xq8_sb = consts.tile([128, CT // 2, 2, S], FP8)
    wq_sb = consts.tile([128, CT, 2 * 128], BF16)
    wk_sb = consts.tile([128, CT, 2 * 128], BF16)
    wv_sb = consts.tile([128, CT, 2 * 128], BF16)
    wo_sb = consts.tile([128, PAIRS, HIDDEN], BF16)
    bq_sb = consts.tile([128, PAIRS, 1], F32)
    bk_sb = consts.tile([128, PAIRS, 1], F32)
    bv_sb = consts.tile([128, PAIRS, 1], F32)

    qT = consts.tile([128, PAIRS, S], BF16)
    kT = consts.tile([128, PAIRS, S], BF16)
    # [V_h | 1] in bf16: stationary [128, 65] per (kt, h); col 64 = 1.0 is
    # the fused softmax-denominator column (fp8 DoubleRow ctx measured
    # ~3e-2 rel err -- above the 2e-2 gate -- so ctx stays bf16).
    vk = consts.tile([128, NKT, HPC, 65], BF16)
    nc.vector.memset(vk, 1.0)
    ctxT2 = consts.tile([128, PAIRS, S], BF16)

    # ---- loads: sync queue feeds q-proj (wq+xb chunk-pipelined);
    # the scalar engine's DMA queue pulls the rest in parallel.
    wq_r = wq_d.rearrange("(t p) m -> p t m", p=128)
    xb_r = xb_d.rearrange("(t p) m -> p t m", p=128)
    nc.sync.dma_start(out=bq_sb, in_=bq_d.rearrange("(j p) one -> p j one", p=128))
    for c in range(CT):
        nc.sync.dma_start(out=wq_sb[:, c, :], in_=wq_r[:, c, :])
        nc.sync.dma_start(out=xb_sb[:, c, :], in_=xb_r[:, c, :])
    nc.scalar.dma_start(out=wk_sb, in_=wk_d.rearrange("(t p) m -> p t m", p=128))
    nc.scalar.dma_start(out=bk_sb, in_=bk_d.rearrange("(j p) one -> p j one", p=128))
    nc.scalar.dma_start(out=wv_sb, in_=wv_d.rearrange("(t p) m -> p t m", p=128))
    nc.scalar.dma_start(out=bv_sb, in_=bv_d.rearrange("(j p) one -> p j one", p=128))
    nc.scalar.dma_start(
        out=xq8_sb,
        in_=xq8_d.rearrange("(c two p) m -> p c two m", c=CT // 2, two=2))
    nc.scalar.dma_start(out=wo_sb, in_=wo_d.rearrange("(j p) o -> p j o", p=128))

    # ---- phase 1: projections ----
    # q-projection runs contraction-outer over 8 live psum tiles so each
    # matmul only needs one xb chunk -> overlaps the xb DMA.
    with tc.tile_pool(name="qprojps", bufs=1, space="PSUM") as qprojps:
        qps = [qprojps.tile([128, QB], F32, tag=f"qp{j}_{nb}",
                            name=f"qp_{j}_{nb}")
               for j in range(PAIRS) for nb in range(S // QB)]
        for c in range(CT):
            for j in range(PAIRS):
                for nb in range(S // QB):
                    nc.tensor.matmul(
                        qps[j * (S // QB) + nb],
                        wq_sb[:, c, j * 128:(j + 1) * 128],
                        xb_sb[:, c, nb * QB:(nb + 1) * QB],
                        start=(c == 0),
                        stop=(c == CT - 1),
                    )
        for j in range(PAIRS):
            for nb in range(S // QB):
                nc.scalar.activation(
                    out=qT[:, j, nb * QB:(nb + 1) * QB],
                    in_=qps[j * (S // QB) + nb],
                    func=ACT_IDENT, bias=bq_sb[:, j, :])

    with tc.tile_pool(name="projps", bufs=2, space="PSUM") as projps:
        for j in range(PAIRS):
            for nb in range(S // QB):
                ps = projps.tile([128, QB], F32, tag="prj",
                                 name=f"prj_k_{j}_{nb}")
                for c in range(CT):
                    nc.tensor.matmul(
                        ps,
                        wk_sb[:, c, j * 128:(j + 1) * 128],
                        xb_sb[:, c, nb * QB:(nb + 1) * QB],
                        start=(c == 0),
                        stop=(c == CT - 1),
                    )
                nc.scalar.activation(
                    out=kT[:, j, nb * QB:(nb + 1) * QB], in_=ps,
                    func=ACT_IDENT, bias=bk_sb[:, j, :])
        # V directly in [keys, d] layout: contraction over hidden with
        # xb as stationary (keys = output partitions). bv folds into the
        # host-side output bias. Output cast to fp8 for DoubleRow ctx.
        for kt in range(NKT):
            ps = projps.tile([128, 2 * 128], F32, tag="prjv",
                             name=f"prj_v_{kt}")
            for c in range(CT):
                nc.tensor.matmul(
                    ps,
                    xb_sb[:, c, kt * 128:(kt + 1) * 128],
                    wv_sb[:, c, :],
                    start=(c == 0),
                    stop=(c == CT - 1),
                )
            nc.scalar.activation(
                out=vk[:, kt, :, 0:HEAD_DIM],
                in_=ps.rearrange("p (h d) -> p h d", h=HPC),
                func=ACT_COPY)

    # ---- phase 2: attention main loop ----
    simp = stack.enter_context(tc.tile_pool(name="simp", bufs=1, space="PSUM"))
    scp = stack.enter_context(tc.tile_pool(name="scp", bufs=1, space="PSUM"))
    ctxp = stack.enter_context(tc.tile_pool(name="ctxp", bufs=1, space="PSUM"))
    outp = stack.enter_context(tc.tile_pool(name="outp", bufs=1, space="PSUM"))

    ep = stack.enter_context(tc.tile_pool(name="ep", bufs=4))
    pexpp = stack.enter_context(tc.tile_pool(name="pexpp", bufs=4))
    ptp = stack.enter_context(tc.tile_pool(name="ptp", bufs=14))
    stagep = stack.enter_context(tc.tile_pool(name="stagep", bufs=3))
    r0p = stack.enter_context(tc.tile_pool(name="r0p", bufs=2))
    rbp = stack.enter_context(tc.tile_pool(name="rbp", bufs=2))
    dstgp = stack.enter_context(tc.tile_pool(name="dstgp", bufs=2))
    mp = (stack.enter_context(tc.tile_pool(name="mp", bufs=2))
          if mask_d is not None else None)
    msp = (stack.enter_context(tc.tile_pool(name="msp", bufs=2))
           if mask_d is not None else None)

    def emit_ctx_head(ctxs, ktp, h, pt):
        # [V_h | 1] stationary: each matmul produces ctx (rows 0-63) plus
        # the softmax denominator (row 64) from a single pt stream.
        for i in range(2):
            nc.tensor.matmul(
                ctxs[h],
                vk[:, 2 * ktp + i, h, :],
                pt[:, i, :],
                start=(ktp == 0 and i == 0),
                stop=(ktp == NKTP - 1 and i == 1),
                skip_group_check=True,
            )

    def emit_division_recips(qb0):
        # Phase A (DVE): reciprocal of each head's fused sums row
        # (partition 64 of its ctx bank), then a bf16 copy so the K=1
        # broadcast matmul runs at 1 cyc/col instead of fp32's 4.
        ctxs = qstate[qb0]
        r16s = []
        for h in range(HPC):
            # full-tile recip: the custom-DVE op mishandles partition-offset
            # APs, so compute junk recips on rows 0-63 too (never read).
            r = r0p.tile([65, QB], F32, tag="r0", name=f"r_{qb0}_{h}")
            nc.vector.reciprocal_approx_fast(out=r, in_=ctxs[h])
            r16 = rbp.tile([65, QB], BF16, tag="rb", name=f"r16_{qb0}_{h}")
            nc.vector.tensor_copy(r16[64:65, :], r[64:65, :])
            r16s.append(r16)
        return r16s

    def emit_division_apply(qb0, r16s):
        # Phase B: K=1 broadcast matmul to partitions 0-63 (DR forbade a
        # col-tiled dst, so every ctx head lives at partition 0), DVE
        # divide-multiply. Odd heads go via a stage tile + identity
        # matmul to shift onto partitions 64-127 of ctxT2.
        ctxs = qstate[qb0]
        qsl0 = slice(qb0 * QB, (qb0 + 1) * QB)
        for h in range(HPC):
            j, hi = divmod(h, 2)
            ctx_h = ctxs[h]
            rb_ps = outp.tile([128, QB], F32, tag="op",
                              name=f"rbps_{qb0}_{h}")
            nc.tensor.matmul(
                rb_ps[0:64, :],
                ones64[64:65, 0:64],
                r16s[h][64:65, :],
                start=True, stop=True,
                tile_position=(64, 0),
            )
            # DVE can read only one PSUM operand: stage rb in SBUF (bf16
            # is lossless here, the values are already bf16-rounded).
            rb = rbp.tile([64, QB], BF16, tag="rbb", name=f"rb_{qb0}_{h}")
            nc.vector.tensor_copy(rb, rb_ps[0:64, :])
            if hi == 0:
                nc.vector.tensor_mul(ctxT2[0:64, j, qsl0],
                                     ctx_h[0:64, :], rb)
            else:
                stg = dstgp.tile([64, QB], BF16, tag="dstg",
                                 name=f"dstg_{qb0}_{h}")
                nc.vector.tensor_mul(stg, ctx_h[0:64, :], rb)
                sh_ps = outp.tile([128, QB], F32, tag="op",
                                  name=f"shps_{qb0}_{h}")
                nc.tensor.matmul(
                    sh_ps[64:128, :], ident64, stg,
                    start=True, stop=True, tile_position=(0, 64),
                )
                nc.vector.tensor_copy(ctxT2[64:128, j, qsl0],
                                      sh_ps[64:128, :])
        div_done[qb0] = True

    def emit_outproj_tile(qb0, i):
        qt = qb0 * (QB // 128) + i // 2
        ob = i % 2
        op = outp.tile([128, 512], F32, tag="op", name=f"op_{qb0}_{i}")
        for j in range(PAIRS):
            nc.tensor.matmul(
                op,
                ctxT2[:, j, qt * 128:(qt + 1) * 128],
                wo_sb[:, j, ob * 512:(ob + 1) * 512],
                start=(j == 0),
                stop=(j == PAIRS - 1),
            )
        st = stagep.tile([128, 512], F32, tag="st", name=f"st_{qb0}_{i}")
        if i % 2 == 0:
            nc.vector.tensor_copy(st, op)
        else:
            nc.scalar.activation(out=st, in_=op, func=ACT_COPY)
        nc.sync.dma_start(
            out=out_d[qt * 128:(qt + 1) * 128, ob * 512:(ob + 1) * 512],
            in_=st)

    LAGK = 4  # head-granular lag of ctx emission behind pt production
    pending = []   # (qb, ktp, h, pt8)
    qstate = {}    # qb -> [ctx_h tiles]

    def get_qstate(qb0):
        if qb0 not in qstate:
            qstate[qb0] = [ctxp.tile([65, QB], F32, tag=f"ctx{h}",
                                     name=f"ctx_{qb0}_{h}")
                           for h in range(HPC)]
        return qstate[qb0]

    div_done = {-1: True}

    def pop_ok():
        return pending and (pending[0][0] - 1) in div_done

    def pop_pending():
        qb0, ktp0, h0, pt0 = pending.pop(0)
        emit_ctx_head(get_qstate(qb0), ktp0, h0, pt0)

    def emit_E_half(Ep, qb, ktp, i, sp):
        if mask_d is None:
            nc.scalar.activation(out=Ep[:, i, :], in_=sp, func=ACT_EXP,
                                 scale=-GAMMA / XQ8_SCALE ** 2)
        else:
            kt = 2 * ktp + i
            ksl = slice(kt * 128, (kt + 1) * 128)
            qsl = slice(qb * QB, (qb + 1) * QB)
            m_sb = mp.tile([128, QB], BF16, tag="m")
            nc.sync.dma_start(out=m_sb, in_=mask_d[ksl, qsl])
            ms = msp.tile([128, QB], BF16, tag="ms")
            nc.vector.scalar_tensor_tensor(
                out=ms, in0=sp, scalar=-GAMMA / XQ8_SCALE ** 2,
                in1=m_sb, op0=ALU.mult, op1=ALU.subtract)
            nc.scalar.activation(out=Ep[:, i, :], in_=ms, func=ACT_EXP)

    def emit_sim(qb, kt):
        qsl = slice(qb * QB, (qb + 1) * QB)
        ksl = slice(kt * 128, (kt + 1) * 128)
        sp = simp.tile([128, QB], F32, tag="sim", name=f"sim_{qb}_{kt}")
        for c in range(CT // 2):
            nc.tensor.matmul(sp, xq8_sb[:, c, :, ksl],
                             xq8_sb[:, c, :, qsl],
                             start=(c == 0), stop=(c == CT // 2 - 1),
                             perf_mode=DR)
        return sp

    for qb in range(NQB):
        qsl = slice(qb * QB, (qb + 1) * QB)
        for ktp in range(NKTP):
            div_r16s = None
            if ktp == 0 and qb > 0:
                # previous block fully decided: drain its pops, then start
                # the division reciprocals on the DVE while the PE rolls
                # straight into this block's sims.
                while pending and pending[0][0] == qb - 1:
                    pop_pending()
                div_r16s = emit_division_recips(qb - 1)
            if ktp > 0:
                while len(pending) > LAGK and pop_ok():
                    pop_pending()
            # sim for the first kt of the pair; the second is staggered
            # into the head loop so the single sim buffer can recycle
            # through the E activation without stalling the PE.
            Ep = ep.tile([128, 2, QB], BF16, tag="E", name=f"E_{qb}_{ktp}")
            sp = emit_sim(qb, 2 * ktp)
            emit_E_half(Ep, qb, ktp, 0, sp)
            if div_r16s is not None:
                emit_division_apply(qb - 1, div_r16s)
            for h in range(HPC):
                j, hi = divmod(h, 2)
                pr = slice(hi * 64, hi * 64 + 64)
                sc = scp.tile([128, 2, QB], F32, tag="sc",
                              name=f"sc_{qb}_{ktp}_{h}")
                for i in range(2):
                    kt = 2 * ktp + i
                    ksl = slice(kt * 128, (kt + 1) * 128)
                    nc.tensor.matmul(sc[:, i, :], kT[pr, j, ksl],
                                     qT[pr, j, qsl], start=True, stop=True)
                # keep the PE fed while pexp drains the single sc buffer:
                # pops + sim_b (h0) / outproj (h1) / double-pop (h2); h3's
                # gap is covered by the next block's sim_a.
                if len(pending) > LAGK and pop_ok():
                    pop_pending()
                if h == 0:
                    sp = emit_sim(qb, 2 * ktp + 1)
                    emit_E_half(Ep, qb, ktp, 1, sp)
                if h == 1 and qb > 0:
                    emit_outproj_tile(qb - 1, ktp)
                if h == 2 and len(pending) > LAGK and pop_ok():
                    pop_pending()
                pexp = pexpp.tile([128, 2, QB], BF16, tag="pexp",
                                  name=f"pexp_{qb}_{ktp}_{h}")
                nc.scalar.activation(out=pexp, in_=sc, func=ACT_EXP)
                pt = ptp.tile([128, 2, QB], BF16, tag="pt",
                              name=f"pt_{qb}_{ktp}_{h}")
                nc.vector.tensor_mul(pt, pexp, Ep)
                pending.append((qb, ktp, h, pt))

    # tail: flush, then last block's division + out-projection.
    while pending:
        pop_pending()
    qf = NQB - 1
    r16s = emit_division_recips(qf)
    emit_division_apply(qf, r16s)
    for i in range(2 * (QB // 128)):
        emit_outproj_tile(qf, i)

    stack.close()


def build_nc(*, with_mask=False, enable_asserts=False):
    nc = bacc.Bacc(
        "TRN2", target_bir_lowering=False, debug=False,
        enable_asserts=enable_asserts,
    )
    D2 = HPC * HEAD_DIM
    aps = {}
    aps["xb"] = nc.dram_tensor("xb", [HIDDEN, S], BF16, kind="ExternalInput").ap()
    aps["xq8"] = nc.dram_tensor("xq8", [HIDDEN, S], FP8,
                                kind="ExternalInput").ap()
    for n in ("wq", "wk", "wv"):
        aps[n] = nc.dram_tensor(n, [HIDDEN, D2], BF16, kind="ExternalInput").ap()
    aps["wo"] = nc.dram_tensor("wo", [D2, HIDDEN], BF16, kind="ExternalInput").ap()
    for n in ("bq", "bk", "bv"):
        aps[n] = nc.dram_tensor(n, [D2, 1], F32, kind="ExternalInput").ap()
    if with_mask:
        aps["maskadd"] = nc.dram_tensor(
            "maskadd", [S, S], BF16, kind="ExternalInput").ap()
    aps["out"] = nc.dram_tensor("out", [S, HIDDEN], F32,
                                kind="ExternalOutput").ap()

    with tile.TileContext(nc) as tc:
        emit_kernel(tc, aps)
    nc.compile()
    return nc


def host_prepare(x, attn_mask, Wq, bq, Wk, bk, Wv, bv, Wo, bo):
    """Build the per-core input maps. Returns (in_maps, with_mask)."""
    x = np.asarray(x, np.float32)
    B_ = x.shape[0]
    groups = N_CORES // B_
    Wq = np.asarray(Wq, np.float32); Wk = np.asarray(Wk, np.float32)
    Wv = np.asarray(Wv, np.float32); Wo = np.asarray(Wo, np.float32)
    bq = np.asarray(bq, np.float32); bk = np.asarray(bk, np.float32)
    bv = np.asarray(bv, np.float32)

    inv_sqrt_d = np.float32(1.0 / math.sqrt(HEAD_DIM))
    bf = ml_dtypes.bfloat16
    WqT = np.ascontiguousarray((Wq * inv_sqrt_d).T.astype(bf))
    WkT = np.ascontiguousarray(Wk.T.astype(bf))
    WvT = np.ascontiguousarray(Wv.T.astype(bf))
    WoT = np.ascontiguousarray(Wo.T.astype(bf))
    bq = bq * inv_sqrt_d

    mask = np.asarray(attn_mask)
    with_mask = bool(mask.any())
    maskadd = None
    if with_mask:
        maskadd = np.ascontiguousarray(
            (mask.T.astype(np.float32) * MASK_BIG).astype(bf))

    in_maps = []
    per_batch = {}
    for b in range(B_):
        xbat = x[b]
        norms = np.linalg.norm(xbat, axis=1, keepdims=True)
        xhat = xbat / np.maximum(norms, 1e-12)
        per_batch[b] = (
            np.ascontiguousarray(xbat.T.astype(bf)),
            np.ascontiguousarray(
                (xhat.T * XQ8_SCALE).astype(ml_dtypes.float8_e4m3)),
        )
    for core in range(N_CORES):
        b, g = divmod(core, groups)
        xbT, xq8T = per_batch[b]
        ch = slice(g * HPC * HEAD_DIM, (g + 1) * HPC * HEAD_DIM)
        m = {
            "xb": xbT,
            "xq8": xq8T,
            "wq": np.ascontiguousarray(WqT[:, ch]),
            "wk": np.ascontiguousarray(WkT[:, ch]),
            "wv": np.ascontiguousarray(WvT[:, ch]),
            "wo": np.ascontiguousarray(WoT[ch, :]),
            "bq": np.ascontiguousarray(bq[ch]).reshape(-1, 1),
            "bk": np.ascontiguousarray(bk[ch]).reshape(-1, 1),
            "bv": np.ascontiguousarray(bv[ch]).reshape(-1, 1),
        }
        if with_mask:
            m["maskadd"] = maskadd
        in_maps.append(m)
    return in_maps, with_mask


_NC_CACHE = {}


def _get_nc(with_mask):
    key = with_mask
    if key not in _NC_CACHE:
        _NC_CACHE[key] = build_nc(with_mask=with_mask)
    return _NC_CACHE[key]


LAST_RESULTS = None


def kernel(**inputs):
    global LAST_RESULTS
    in_maps, with_mask = host_prepare(
        inputs["x"], inputs["attn_mask"],
        inputs["Wq"], inputs["bq"], inputs["Wk"], inputs["bk"],
        inputs["Wv"], inputs["bv"], inputs["Wo"], inputs["bo"],
    )
    nc = _get_nc(with_mask)
    res = run_bass_kernel_spmd(nc, in_maps, core_ids=list(range(N_CORES)))
    LAST_RESULTS = res
    bo = np.asarray(inputs["bo"], np.float32)
    bv_full = np.asarray(inputs["bv"], np.float32)
    Wo_full = np.asarray(inputs["Wo"], np.float32)
    bo = bo + bv_full @ Wo_full.T
    out = np.zeros((B, S, HIDDEN), np.float32)
    groups = N_CORES // B
    for core in range(N_CORES):
        b = core // groups
        out[b] += res.results[core]["out"]
    out += bo[None, None, :]
    return out

